# revision 3
# baseline (speedup 1.0000x reference)
"""Sharded RoPE causal attention for 8 Trainium2 NeuronCores.

Problem: B=2, S=2048, E=1024, H=16 heads, D=64 head_dim.
Sharding: batch x head-group (2 batches x 4 groups of 4 heads = 8 cores).
Each core computes its batch's attention for its 4 heads and a partial
output projection (row-parallel Wo); the host sums the 4 partials per batch.

Per-core layout strategy (all matmul layouts prepped on host):
  - x is fed transposed (xT [E, S]) so QKV projections contract E on
    partitions directly.
  - q,k are computed transposed (qT [256, S]) and RoPE is applied in that
    layout. Wq/Wk rows are pre-permuted per head into [even dims | odd dims]
    32-blocks so the RoPE pair partner is a fixed 32-partition block swap,
    done on the PE with a permutation matmul.
  - scores are computed transposed (sT[j,i] = k_j . q_i) per 128-row j-tile,
    causally skipping fully-masked column ranges; the diagonal 128x128 block
    gets -1e30 added below the diagonal before exp.
  - softmax denominators come for free from an appended ones-column on v
    (row 64 of the attn@v output is sum_j exp); normalization happens at
    eviction via gpsimd partition-broadcast + fast reciprocal + multiply.
  - exp is exp(s - C) with a global constant C picked on the host from a
    sampled probe of the score scale (C=0 for typical magnitudes); this is
    mathematically exact (cancels in softmax) and keeps fp32 exp in range.
"""

import sys

for _p in ("/opt/trn_rl_repo",):
    if _p not in sys.path:
        sys.path.insert(0, _p)

import numpy as np

B, S, E, H, D = 2, 2048, 1024, 16, 64
HL = 4          # heads per core
EL = HL * D     # 256: per-core slice of E
N_CORES = 8
NEG = -1e30

_module_cache = {}


def _patch_tile_drain():
    """This toolchain's walrus encodes at most 1 sem wait per instruction;
    Tile's closing drain carries one wait per used logical proc. Split the
    extra waits onto chained SP drains. (Compute-instruction waits are
    split by Bacc.generate_event_semaphores.)"""
    import concourse.tile as tile
    from concourse.vector_clock import ScopedClock

    if getattr(tile.TileContext, "_drain_split_patched", False):
        return

    def _drain_and_barrier(self, tick_clock, wait_clock):
        drain_inst = self.nc.sync.drain()
        wait_clock.add_sem_waits(
            drain_inst.ins, ScopedClock({None: tick_clock.global_clock})
        )
        si = drain_inst.ins.sync_info
        if si is not None and si.on_wait and len(si.on_wait) > 1:
            waits = list(si.on_wait)
            si.on_wait = waits[:1]
            for w in waits[1:]:
                extra = self.nc.sync.drain()
                xsi = extra.ins.sync_info
                if xsi is None:
                    import concourse.mybir as mybir

                    extra.ins.sync_info = mybir.SyncInfo(on_wait=[w], on_update=[])
                else:
                    xsi.on_wait = [w]
        self.nc.all_engine_barrier()
        assert self.sems is not None
        popped = self.nc._tile_sem_poison_stack.pop()
        assert popped is self._sem_poison
        self.nc.clear_and_free_semaphores(list(self.sems.allocated().values()))
        self.nc.all_engine_barrier()

    tile.TileContext._drain_and_barrier = _drain_and_barrier
    tile.TileContext._drain_split_patched = True


def build_module(causal: bool, c_bias: float):
    """Build the per-core Bass module (SPMD: same program on all 8 cores)."""
    _patch_tile_drain()
    from contextlib import ExitStack

    import concourse.tile as tile
    import concourse.mybir as mybir
    from concourse import bacc

    F32 = mybir.dt.float32
    AF = mybir.ActivationFunctionType
    OP = mybir.AluOpType

    nc = bacc.Bacc()

    XT_d = nc.dram_tensor("XT", [E, S], F32, kind="ExternalInput")
    WQT_d = nc.dram_tensor("WQT", [E, EL], F32, kind="ExternalInput")
    WKT_d = nc.dram_tensor("WKT", [E, EL], F32, kind="ExternalInput")
    WVT_d = nc.dram_tensor("WVT", [E, EL], F32, kind="ExternalInput")
    WOT_d = nc.dram_tensor("WOT", [EL, E], F32, kind="ExternalInput")
    CC_d = nc.dram_tensor("CC", [128, S], F32, kind="ExternalInput")
    SS_d = nc.dram_tensor("SS", [128, S], F32, kind="ExternalInput")
    PM_d = nc.dram_tensor("PM", [128, 128], F32, kind="ExternalInput")
    TRI_d = nc.dram_tensor("TRI", [128, 128], F32, kind="ExternalInput")
    OUT_d = nc.dram_tensor("OUT", [S, E], F32, kind="ExternalOutput")

    NST = S // 128   # 16 s-tiles / j-tiles
    NEC = E // 128   # 8 e-chunks

    with tile.TileContext(nc) as tc, ExitStack() as ctx:
        consts = ctx.enter_context(tc.tile_pool(name="consts", bufs=1))
        CC = consts.tile([128, S], F32)
        nc.sync.dma_start(out=CC[:], in_=CC_d[:])
        SS = consts.tile([128, S], F32)
        nc.sync.dma_start(out=SS[:], in_=SS_d[:])
        PM = consts.tile([128, 128], F32)
        nc.sync.dma_start(out=PM[:], in_=PM_d[:])
        TRI = consts.tile([128, 128], F32)
        nc.sync.dma_start(out=TRI[:], in_=TRI_d[:])
        WOT = consts.tile([128, 2, E], F32)
        nc.sync.dma_start(out=WOT[:], in_=WOT_d.rearrange("(c p) e -> p c e", p=128))
        ebias = consts.tile([128, 1], F32)
        nc.vector.memset(ebias[:], -float(c_bias))

        qk = ctx.enter_context(tc.tile_pool(name="qk", bufs=1))
        QT = [qk.tile([128, S], F32, tag=f"qt{t}", name=f"qt{t}") for t in range(2)]
        KT = [qk.tile([128, S], F32, tag=f"kt{t}", name=f"kt{t}") for t in range(2)]
        VA = qk.tile([128, NST, HL, D + 1], F32, tag="vaug")
        nc.vector.memset(VA[:, :, :, D : D + 1], 1.0)
        AN = [qk.tile([128, S], F32, tag=f"an{t}", name=f"an{t}") for t in range(2)]

        # ---------------- Phase 1: projections + RoPE ----------------
        with tc.tile_pool(name="xw", bufs=1) as xw, \
             tc.tile_pool(name="p1ps", bufs=3, space="PSUM") as p1ps, \
             tc.tile_pool(name="p1vps", bufs=2, space="PSUM") as p1vps, \
             tc.tile_pool(name="p1sw", bufs=2, space="PSUM") as p1sw, \
             tc.tile_pool(name="p1sb", bufs=2) as p1sb, \
             tc.tile_pool(name="p1t1", bufs=2) as p1t1:
            XTs = xw.tile([128, NEC, S], F32, tag="xt")
            nc.sync.dma_start(out=XTs[:], in_=XT_d.rearrange("(c p) s -> p c s", p=128))
            WQTs = xw.tile([128, NEC, EL], F32, tag="wq")
            nc.sync.dma_start(out=WQTs[:], in_=WQT_d.rearrange("(c p) j -> p c j", p=128))
            WKTs = xw.tile([128, NEC, EL], F32, tag="wk")
            nc.sync.dma_start(out=WKTs[:], in_=WKT_d.rearrange("(c p) j -> p c j", p=128))
            WVTs = xw.tile([128, NEC, EL], F32, tag="wv")
            nc.sync.dma_start(out=WVTs[:], in_=WVT_d.rearrange("(c p) j -> p c j", p=128))

            # q/k transposed projections + RoPE (chunked along s)
            for wten, dest in ((WQTs, QT), (WKTs, KT)):
                for t in range(2):
                    for sc in range(4):
                        cs = slice(sc * 512, (sc + 1) * 512)
                        ps = p1ps.tile([128, 512], F32, tag="pqk")
                        for ec in range(NEC):
                            nc.tensor.matmul(
                                ps[:],
                                wten[:, ec, t * 128 : (t + 1) * 128],
                                XTs[:, ec, cs],
                                start=(ec == 0),
                                stop=(ec == NEC - 1),
                            )
                        raw = p1sb.tile([128, 512], F32, tag="raw", bufs=3)
                        nc.scalar.copy(raw[:], ps[:])
                        t1 = p1t1.tile([128, 512], F32, tag="t1")
                        nc.vector.tensor_mul(t1[:], raw[:], CC[:, cs])
                        sw = p1sw.tile([128, 512], F32, tag="psw")
                        nc.tensor.matmul(sw[:], PM[:], raw[:], start=True, stop=True)
                        t2 = p1sb.tile([128, 512], F32, tag="t2")
                        nc.vector.tensor_mul(t2[:], sw[:], SS[:, cs])
                        nc.vector.tensor_add(dest[t][:, cs], t1[:], t2[:])

            # v natural projection into VA
            for st in range(NST):
                pv = p1vps.tile([128, EL], F32, tag="pv")
                for ec in range(NEC):
                    nc.tensor.matmul(
                        pv[:],
                        XTs[:, ec, st * 128 : (st + 1) * 128],
                        WVTs[:, ec, :],
                        start=(ec == 0),
                        stop=(ec == NEC - 1),
                    )
                nc.scalar.copy(
                    VA[:, st, :, 0:D],
                    pv[:].rearrange("p (h d) -> p h d", h=HL),
                )

        # ---------------- Phase 2: attention ----------------
        with tc.tile_pool(name="sps", bufs=2, space="PSUM") as sps, \
             tc.tile_pool(name="aops", bufs=2, space="PSUM") as aops, \
             tc.tile_pool(name="ssb", bufs=3) as ssb, \
             tc.tile_pool(name="nrm", bufs=2) as nrm:
            for h in range(HL):
                t = h // 2
                r0 = 64 * (h % 2)
                for half in range(2):
                    i0 = half * 1024
                    jjs = list(range(8 * (half + 1))) if causal else list(range(NST))
                    pao = aops.tile([65, 1024], F32, tag="pao")
                    # per-bank last-writer for stop flags
                    lastA = max(jj for jj in jjs
                                if (max(0, 128 * jj - i0) if causal else 0) < 512)
                    lastB = jjs[-1]
                    for jj in jjs:
                        off = max(0, 128 * jj - i0) if causal else 0
                        pieces = [(off, 512), (512, 1024)] if off < 512 else [(off, 1024)]
                        sp = sps.tile([128, 1024], F32, tag="sp")
                        for (a, b) in pieces:
                            nc.tensor.matmul(
                                sp[:, a:b],
                                KT[t][r0 : r0 + 64, jj * 128 : (jj + 1) * 128],
                                QT[t][r0 : r0 + 64, i0 + a : i0 + b],
                                start=True,
                                stop=True,
                            )
                        if causal and 8 * half <= jj < 8 * (half + 1):
                            nc.vector.tensor_add(
                                sp[:, off : off + 128],
                                sp[:, off : off + 128],
                                TRI[:],
                            )
                        st_sb = ssb.tile([128, 1024], F32, tag="st")
                        nc.scalar.activation(
                            st_sb[:, off:1024], sp[:, off:1024], AF.Exp,
                            bias=ebias[:], scale=1.0,
                        )
                        for (a, b) in pieces:
                            nc.tensor.matmul(
                                pao[0:65, a:b],
                                VA[:, jj, h, :],
                                st_sb[:, a:b],
                                start=(jj == 0),
                                stop=(jj == (lastA if b == 512 else lastB)),
                            )
                    # normalize + evict into AN
                    u = nrm.tile([65, 1024], F32, tag="u")
                    nc.scalar.copy(u[:], pao[:])
                    d0 = nrm.tile([1, 1024], F32, tag="d0")
                    nc.vector.tensor_copy(d0[0:1, :], u[64:65, :])
                    bc = nrm.tile([64, 1024], F32, tag="bc")
                    nc.gpsimd.partition_broadcast(bc[:], d0[0:1, :], channels=64)
                    inv = nrm.tile([64, 1024], F32, tag="inv")
                    nc.vector.reciprocal_approx_fast(inv[:], bc[:])
                    nc.vector.tensor_mul(
                        AN[t][r0 : r0 + 64, i0 : i0 + 1024], u[0:64, :], inv[:]
                    )

        # ---------------- Phase 3: output projection ----------------
        with tc.tile_pool(name="ops", bufs=2, space="PSUM") as ops, \
             tc.tile_pool(name="osb", bufs=3) as osb:
            for st in range(NST):
                po = ops.tile([128, E], F32, tag="po")
                for eh in range(2):
                    for p in range(2):
                        nc.tensor.matmul(
                            po[:, eh * 512 : (eh + 1) * 512],
                            AN[p][:, st * 128 : (st + 1) * 128],
                            WOT[:, p, eh * 512 : (eh + 1) * 512],
                            start=(p == 0),
                            stop=(p == 1),
                        )
                ob = osb.tile([128, E], F32, tag="ob")
                if st % 2 == 0:
                    nc.scalar.copy(ob[:], po[:])
                else:
                    nc.vector.tensor_copy(ob[:], po[:])
                nc.sync.dma_start(out=OUT_d[st * 128 : (st + 1) * 128, :], in_=ob[:])

    nc.compile()
    return nc


def _get_module(causal: bool, c_bias: float):
    key = (causal, round(float(c_bias), 3))
    if key not in _module_cache:
        _module_cache[key] = build_module(causal, c_bias)
    return _module_cache[key]


_PERM64 = np.concatenate([np.arange(0, 64, 2), np.arange(1, 64, 2)])


def host_prep(x, sin_emb, cos_emb, Wq, Wk, Wv, Wo):
    """Build per-core input maps (host-side sharding + layout prep)."""
    x = np.asarray(x, np.float32)
    sin_emb = np.asarray(sin_emb, np.float32)
    cos_emb = np.asarray(cos_emb, np.float32)
    Wq = np.asarray(Wq, np.float32)
    Wk = np.asarray(Wk, np.float32)
    Wv = np.asarray(Wv, np.float32)
    Wo = np.asarray(Wo, np.float32)

    xT = [np.ascontiguousarray(x[b].T) for b in range(B)]

    cosT = np.ascontiguousarray(cos_emb.T)   # [32, S]
    sinT = np.ascontiguousarray(sin_emb.T)
    CC = np.ascontiguousarray(np.tile(cosT, (4, 1)))
    SSm = np.ascontiguousarray(
        np.concatenate([-sinT, sinT, -sinT, sinT], axis=0)
    )
    PM = np.zeros((128, 128), np.float32)
    for p in range(128):
        q = p + 32 if (p % 64) < 32 else p - 32
        PM[p, q] = 1.0
    TRIm = np.where(
        np.arange(128)[:, None] > np.arange(128)[None, :], np.float32(NEG), np.float32(0)
    ).astype(np.float32)

    scale = np.float32(1.0 / np.sqrt(D))
    in_maps = []
    for c in range(N_CORES):
        b, g = divmod(c, HL)
        rows = np.concatenate([(4 * g + lh) * 64 + _PERM64 for lh in range(HL)])
        in_maps.append({
            "XT": xT[b],
            "WQT": np.ascontiguousarray((Wq[rows, :] * scale).T),
            "WKT": np.ascontiguousarray(Wk[rows, :].T),
            "WVT": np.ascontiguousarray(Wv[256 * g : 256 * (g + 1), :].T),
            "WOT": np.ascontiguousarray(Wo[:, 256 * g : 256 * (g + 1)].T),
            "CC": CC,
            "SS": SSm,
            "PM": PM,
            "TRI": TRIm,
        })
    return in_maps


def probe_bias(x, Wq, Wk):
    """Estimate the global score scale on a row sample; returns exp bias C."""
    x = np.asarray(x, np.float32)
    xs = x[:, ::16, :].reshape(-1, E)           # 256 sampled rows
    qs = xs @ np.asarray(Wq, np.float32).T
    ks = xs @ np.asarray(Wk, np.float32).T
    m = 0.0
    for b in range(B):
        qb = qs[b * 128 : (b + 1) * 128].reshape(128, H, D)
        kb = ks[b * 128 : (b + 1) * 128].reshape(128, H, D)
        s = np.einsum("qhd,khd->hqk", qb, kb) / np.sqrt(np.float32(D))
        m = max(m, float(np.abs(s).max()))
    if m * 1.6 < 25.0:
        return 0.0
    return round(m * 1.3, 3)


def classify_mask(mask):
    mask = np.asarray(mask)
    m0 = mask[0, 0]
    ar = np.arange(S)
    tril = (ar[:, None] >= ar[None, :])
    if all((mask[b, 0] != 0).astype(bool).__eq__(tril).all() for b in range(B)):
        return "causal"
    if (mask != 0).all():
        return "full"
    return "other"


def _numpy_fallback(x, sin_emb, cos_emb, mask, Wq, Wk, Wv, Wo):
    x = np.asarray(x, np.float32)
    sin_emb = np.asarray(sin_emb, np.float32)
    cos_emb = np.asarray(cos_emb, np.float32)
    mask = np.asarray(mask)
    Wq, Wk, Wv, Wo = (np.asarray(w, np.float32) for w in (Wq, Wk, Wv, Wo))

    def rope(t):
        sin = sin_emb[None, :, None, :]
        cos = cos_emb[None, :, None, :]
        x1 = t[..., 0::2]
        x2 = t[..., 1::2]
        r0 = x1 * cos - x2 * sin
        r1 = x1 * sin + x2 * cos
        return np.stack((r0, r1), axis=-1).reshape(t.shape)

    q = rope((x @ Wq.T).reshape(B, S, H, D))
    k = rope((x @ Wk.T).reshape(B, S, H, D))
    v = (x @ Wv.T).reshape(B, S, H, D)
    scores = np.einsum("bqhd,bkhd->bhqk", q, k) / np.sqrt(np.float32(D))
    scores = np.where(mask == 0, -np.inf, scores)
    scores -= scores.max(axis=-1, keepdims=True)
    ex = np.exp(scores)
    attn = ex / ex.sum(axis=-1, keepdims=True)
    out = np.einsum("bhqk,bkhd->bqhd", attn, v).reshape(B, S, E)
    return (out @ Wo.T).astype(np.float32)


def kernel(x, sin_emb, cos_emb, mask, Wq, Wk, Wv, Wo):
    mode = classify_mask(mask)
    if mode == "other":
        return _numpy_fallback(x, sin_emb, cos_emb, mask, Wq, Wk, Wv, Wo)

    from concourse.bass_utils import run_bass_kernel_spmd

    c_bias = probe_bias(x, Wq, Wk)
    nc = _get_module(mode == "causal", c_bias)
    in_maps = host_prep(x, sin_emb, cos_emb, Wq, Wk, Wv, Wo)
    res = run_bass_kernel_spmd(nc, in_maps, core_ids=list(range(N_CORES)))
    out = np.zeros((B, S, E), np.float32)
    for c in range(N_CORES):
        b = c // HL
        out[b] += res.results[c]["OUT"]
    return out


# revision 8
# speedup vs baseline: 2.4600x; 2.4600x over previous
"""Sharded RoPE causal attention for 8 Trainium2 NeuronCores.

Problem: B=2, S=2048, E=1024, H=16 heads, D=64 head_dim.
Sharding: batch x head-group (2 batches x 4 groups of 4 heads = 8 cores).
Each core computes its batch's attention for its 4 heads and a partial
output projection (row-parallel Wo); the host sums the 4 partials per batch.

Per-core layout strategy (all matmul layouts prepped on host):
  - x is fed transposed (xT [E, S]) so QKV projections contract E on
    partitions directly.
  - q,k are computed transposed (qT [256, S]) and RoPE is applied in that
    layout. Wq/Wk rows are pre-permuted per head into [even dims | odd dims]
    32-blocks so the RoPE pair partner is a fixed 32-partition block swap,
    done on the PE with a permutation matmul.
  - scores are computed transposed (sT[j,i] = k_j . q_i) per 128-row j-tile,
    causally skipping fully-masked column ranges; the diagonal 128x128 block
    gets -1e30 added below the diagonal before exp.
  - softmax denominators come for free from an appended ones-column on v
    (row 64 of the attn@v output is sum_j exp); normalization happens at
    eviction via gpsimd partition-broadcast + fast reciprocal + multiply.
  - exp is exp(s - C) with a global constant C picked on the host from a
    sampled probe of the score scale (C=0 for typical magnitudes); this is
    mathematically exact (cancels in softmax) and keeps fp32 exp in range.
"""

import sys

for _p in ("/opt/trn_rl_repo",):
    if _p not in sys.path:
        sys.path.insert(0, _p)

import numpy as np

B, S, E, H, D = 2, 2048, 1024, 16, 64
HL = 4          # heads per core
EL = HL * D     # 256: per-core slice of E
N_CORES = 8
NEG = -1e30

_module_cache = {}


def _patch_tile_drain():
    """This toolchain's walrus encodes at most 1 sem wait per instruction;
    Tile's closing drain carries one wait per used logical proc. Split the
    extra waits onto chained SP drains. (Compute-instruction waits are
    split by Bacc.generate_event_semaphores.)"""
    import concourse.tile as tile
    from concourse.vector_clock import ScopedClock

    if getattr(tile.TileContext, "_drain_split_patched", False):
        return

    def _drain_and_barrier(self, tick_clock, wait_clock):
        drain_inst = self.nc.sync.drain()
        wait_clock.add_sem_waits(
            drain_inst.ins, ScopedClock({None: tick_clock.global_clock})
        )
        si = drain_inst.ins.sync_info
        if si is not None and si.on_wait and len(si.on_wait) > 1:
            waits = list(si.on_wait)
            si.on_wait = waits[:1]
            for w in waits[1:]:
                extra = self.nc.sync.drain()
                xsi = extra.ins.sync_info
                if xsi is None:
                    import concourse.mybir as mybir

                    extra.ins.sync_info = mybir.SyncInfo(on_wait=[w], on_update=[])
                else:
                    xsi.on_wait = [w]
        self.nc.all_engine_barrier()
        assert self.sems is not None
        popped = self.nc._tile_sem_poison_stack.pop()
        assert popped is self._sem_poison
        self.nc.clear_and_free_semaphores(list(self.sems.allocated().values()))
        self.nc.all_engine_barrier()

    tile.TileContext._drain_and_barrier = _drain_and_barrier
    tile.TileContext._drain_split_patched = True


def build_module(causal: bool, c_bias: float, use_fp32r: bool = True):
    """Build the per-core Bass module (SPMD: same program on all 8 cores)."""
    _patch_tile_drain()
    from contextlib import ExitStack

    import concourse.tile as tile
    import concourse.mybir as mybir
    from concourse import bacc

    F32 = mybir.dt.float32
    AF = mybir.ActivationFunctionType
    OP = mybir.AluOpType

    nc = bacc.Bacc()

    # fp32 matmul is 4 cycles/row on TRN2; float32r streams at full rate.
    # The BIR verifier requires fp32r matmul operands to be *produced* as
    # float32r, so every matmul-operand tensor below is typed R32 (same bit
    # layout as fp32; numpy side stays float32).
    R32 = mybir.dt.float32r if use_fp32r else F32
    mm = nc.tensor.matmul

    NST = S // 128   # 16 s-tiles / j-tiles
    NEC = E // 128   # 8 e-chunks

    XT_d = nc.dram_tensor("XT", [E, S], R32, kind="ExternalInput")
    WQT_d = nc.dram_tensor("WQT", [E, EL], R32, kind="ExternalInput")
    WKT_d = nc.dram_tensor("WKT", [E, EL], R32, kind="ExternalInput")
    WVT_d = nc.dram_tensor("WVT", [E, EL], R32, kind="ExternalInput")
    WOT_d = nc.dram_tensor("WOT", [EL, E], R32, kind="ExternalInput")
    CC_d = nc.dram_tensor("CC", [128, S], F32, kind="ExternalInput")
    SS_d = nc.dram_tensor("SS", [128, S], F32, kind="ExternalInput")
    PM_d = nc.dram_tensor("PM", [128, 128], R32, kind="ExternalInput")
    TRI_d = nc.dram_tensor("TRI", [128, 128], F32, kind="ExternalInput")
    VONES_d = nc.dram_tensor("VONES", [128, NST * HL], R32, kind="ExternalInput")
    OUT_d = nc.dram_tensor("OUT", [S, E], F32, kind="ExternalOutput")

    with tile.TileContext(nc) as tc, ExitStack() as ctx:
        consts = ctx.enter_context(tc.tile_pool(name="consts", bufs=1))
        CC = consts.tile([128, S], F32)
        nc.sync.dma_start(out=CC[:], in_=CC_d[:])
        SS = consts.tile([128, S], F32)
        nc.sync.dma_start(out=SS[:], in_=SS_d[:])
        PM = consts.tile([128, 128], R32)
        nc.sync.dma_start(out=PM[:], in_=PM_d[:])
        TRI = consts.tile([128, 128], F32)
        nc.sync.dma_start(out=TRI[:], in_=TRI_d[:])
        WOT = consts.tile([128, 2, E], R32)
        nc.sync.dma_start(out=WOT[:], in_=WOT_d.rearrange("(c p) e -> p c e", p=128))
        ebias = consts.tile([128, 1], F32)
        nc.vector.memset(ebias[:], -float(c_bias))

        qk = ctx.enter_context(tc.tile_pool(name="qk", bufs=1))
        QT = [qk.tile([128, S], R32, tag=f"qt{t}", name=f"qt{t}") for t in range(2)]
        KT = [qk.tile([128, S], R32, tag=f"kt{t}", name=f"kt{t}") for t in range(2)]
        VA = qk.tile([128, NST, HL, D + 1], R32, tag="vaug")
        nc.sync.dma_start(
            out=VA[:, :, :, D : D + 1],
            in_=VONES_d.rearrange("p (st h) -> p st h", h=HL),
        )
        AN = [qk.tile([128, S], R32, tag=f"an{t}", name=f"an{t}") for t in range(2)]

        # ---------------- Phase 1: projections + RoPE ----------------
        with tc.tile_pool(name="xw", bufs=1) as xw, \
             tc.tile_pool(name="p1ps", bufs=3, space="PSUM") as p1ps, \
             tc.tile_pool(name="p1vps", bufs=2, space="PSUM") as p1vps, \
             tc.tile_pool(name="p1sw", bufs=2, space="PSUM") as p1sw, \
             tc.tile_pool(name="p1sb", bufs=2) as p1sb, \
             tc.tile_pool(name="p1t1", bufs=2) as p1t1:
            WQTs = xw.tile([128, NEC, EL], R32, tag="wq")
            nc.sync.dma_start(out=WQTs[:], in_=WQT_d.rearrange("(c p) j -> p c j", p=128))
            XTs = xw.tile([128, NEC, S], R32, tag="xt")
            for ec in range(NEC):
                nc.sync.dma_start(
                    out=XTs[:, ec, :], in_=XT_d[ec * 128 : (ec + 1) * 128, :]
                )
            WKTs = xw.tile([128, NEC, EL], R32, tag="wk")
            nc.sync.dma_start(out=WKTs[:], in_=WKT_d.rearrange("(c p) j -> p c j", p=128))
            WVTs = xw.tile([128, NEC, EL], R32, tag="wv")
            nc.sync.dma_start(out=WVTs[:], in_=WVT_d.rearrange("(c p) j -> p c j", p=128))

            # q/k transposed projections + RoPE (chunked along s)
            # order: (q,k) tile 0 first so attention on heads 0/1 can begin,
            # then v, then (q,k) tile 1.
            for wten, dest, t in (
                (WQTs, QT, 0), (WKTs, KT, 0), (None, None, -1),
                (WQTs, QT, 1), (WKTs, KT, 1),
            ):
                if t < 0:
                    # v natural projection into VA
                    for st in range(NST):
                        pv = p1vps.tile([128, EL], F32, tag="pv")
                        for ec in range(NEC):
                            mm(
                                pv[:],
                                XTs[:, ec, st * 128 : (st + 1) * 128],
                                WVTs[:, ec, :],
                                start=(ec == 0),
                                stop=(ec == NEC - 1),
                            )
                        nc.scalar.copy(
                            VA[:, st, :, 0:D],
                            pv[:].rearrange("p (h d) -> p h d", h=HL),
                        )
                    continue
                if True:
                    for sc in range(4):
                        cs = slice(sc * 512, (sc + 1) * 512)
                        ps = p1ps.tile([128, 512], F32, tag="pqk")
                        for ec in range(NEC):
                            mm(
                                ps[:],
                                wten[:, ec, t * 128 : (t + 1) * 128],
                                XTs[:, ec, cs],
                                start=(ec == 0),
                                stop=(ec == NEC - 1),
                            )
                        raw = p1sb.tile([128, 512], R32, tag="raw", bufs=3)
                        nc.scalar.copy(raw[:], ps[:])
                        t1 = p1t1.tile([128, 512], F32, tag="t1")
                        nc.vector.tensor_mul(t1[:], raw[:], CC[:, cs])
                        sw = p1sw.tile([128, 512], F32, tag="psw")
                        mm(sw[:], PM[:], raw[:], start=True, stop=True)
                        t2 = p1sb.tile([128, 512], F32, tag="t2")
                        nc.vector.tensor_mul(t2[:], sw[:], SS[:, cs])
                        nc.vector.tensor_add(dest[t][:, cs], t1[:], t2[:])


        # ---------------- Phase 2: attention ----------------
        with tc.tile_pool(name="sps", bufs=2, space="PSUM") as sps, \
             tc.tile_pool(name="aops", bufs=2, space="PSUM") as aops, \
             tc.tile_pool(name="ssb", bufs=3) as ssb, \
             tc.tile_pool(name="nrm", bufs=2) as nrm:
            for half in range(2):
                for h in range(HL):
                    t = h // 2
                    r0 = 64 * (h % 2)
                    i0 = half * 1024
                    jjs = list(range(8 * (half + 1))) if causal else list(range(NST))
                    pao = aops.tile([65, 1024], F32, tag="pao")
                    # per-bank last-writer for stop flags
                    lastA = max(jj for jj in jjs
                                if (max(0, 128 * jj - i0) if causal else 0) < 512)
                    lastB = jjs[-1]
                    for jj in jjs:
                        off = max(0, 128 * jj - i0) if causal else 0
                        pieces = [(off, 512), (512, 1024)] if off < 512 else [(off, 1024)]
                        sp = sps.tile([128, 1024], F32, tag="sp")
                        for (a, b) in pieces:
                            mm(
                                sp[:, a:b],
                                KT[t][r0 : r0 + 64, jj * 128 : (jj + 1) * 128],
                                QT[t][r0 : r0 + 64, i0 + a : i0 + b],
                                start=True,
                                stop=True,
                            )
                        if causal and 8 * half <= jj < 8 * (half + 1):
                            nc.vector.tensor_add(
                                sp[:, off : off + 128],
                                sp[:, off : off + 128],
                                TRI[:],
                            )
                        st_sb = ssb.tile([128, 1024], R32, tag="st")
                        nc.scalar.activation(
                            st_sb[:, off:1024], sp[:, off:1024], AF.Exp,
                            bias=ebias[:], scale=1.0,
                        )
                        for (a, b) in pieces:
                            mm(
                                pao[0:65, a:b],
                                VA[:, jj, h, :],
                                st_sb[:, a:b],
                                start=(jj == 0),
                                stop=(jj == (lastA if b == 512 else lastB)),
                            )
                    # normalize + evict into AN
                    u = nrm.tile([65, 1024], F32, tag="u")
                    nc.vector.tensor_copy(u[:], pao[:])
                    d0 = nrm.tile([1, 1024], F32, tag="d0")
                    nc.vector.tensor_copy(d0[0:1, :], u[64:65, :])
                    bc = nrm.tile([64, 1024], F32, tag="bc")
                    nc.gpsimd.partition_broadcast(bc[:], d0[0:1, :], channels=64)
                    inv = nrm.tile([64, 1024], F32, tag="inv")
                    nc.vector.reciprocal_approx_fast(inv[:], bc[:])
                    nc.vector.tensor_mul(
                        AN[t][r0 : r0 + 64, i0 : i0 + 1024], u[0:64, :], inv[:]
                    )

        # ---------------- Phase 3: output projection ----------------
        with tc.tile_pool(name="ops", bufs=2, space="PSUM") as ops, \
             tc.tile_pool(name="osb", bufs=3) as osb:
            for st in range(NST):
                po = ops.tile([128, E], F32, tag="po")
                for eh in range(2):
                    for p in range(2):
                        mm(
                            po[:, eh * 512 : (eh + 1) * 512],
                            AN[p][:, st * 128 : (st + 1) * 128],
                            WOT[:, p, eh * 512 : (eh + 1) * 512],
                            start=(p == 0),
                            stop=(p == 1),
                        )
                ob = osb.tile([128, E], F32, tag="ob")
                nc.scalar.copy(ob[:], po[:])
                nc.sync.dma_start(out=OUT_d[st * 128 : (st + 1) * 128, :], in_=ob[:])

    nc.compile()
    return nc


def _get_module(causal: bool, c_bias: float):
    key = (causal, round(float(c_bias), 3))
    if key not in _module_cache:
        _module_cache[key] = build_module(causal, c_bias)
    return _module_cache[key]


_PERM64 = np.concatenate([np.arange(0, 64, 2), np.arange(1, 64, 2)])


def host_prep(x, sin_emb, cos_emb, Wq, Wk, Wv, Wo):
    """Build per-core input maps (host-side sharding + layout prep)."""
    x = np.asarray(x, np.float32)
    sin_emb = np.asarray(sin_emb, np.float32)
    cos_emb = np.asarray(cos_emb, np.float32)
    Wq = np.asarray(Wq, np.float32)
    Wk = np.asarray(Wk, np.float32)
    Wv = np.asarray(Wv, np.float32)
    Wo = np.asarray(Wo, np.float32)

    xT = [np.ascontiguousarray(x[b].T) for b in range(B)]

    cosT = np.ascontiguousarray(cos_emb.T)   # [32, S]
    sinT = np.ascontiguousarray(sin_emb.T)
    CC = np.ascontiguousarray(np.tile(cosT, (4, 1)))
    SSm = np.ascontiguousarray(
        np.concatenate([-sinT, sinT, -sinT, sinT], axis=0)
    )
    PM = np.zeros((128, 128), np.float32)
    for p in range(128):
        q = p + 32 if (p % 64) < 32 else p - 32
        PM[p, q] = 1.0
    TRIm = np.where(
        np.arange(128)[:, None] > np.arange(128)[None, :], np.float32(NEG), np.float32(0)
    ).astype(np.float32)

    scale = np.float32(1.0 / np.sqrt(D))
    in_maps = []
    for c in range(N_CORES):
        b, g = divmod(c, HL)
        rows = np.concatenate([(4 * g + lh) * 64 + _PERM64 for lh in range(HL)])
        in_maps.append({
            "XT": xT[b],
            "WQT": np.ascontiguousarray((Wq[rows, :] * scale).T),
            "WKT": np.ascontiguousarray(Wk[rows, :].T),
            "WVT": np.ascontiguousarray(Wv[256 * g : 256 * (g + 1), :].T),
            "WOT": np.ascontiguousarray(Wo[:, 256 * g : 256 * (g + 1)].T),
            "CC": CC,
            "VONES": np.ones((128, 64), np.float32),
            "SS": SSm,
            "PM": PM,
            "TRI": TRIm,
        })
    return in_maps


def probe_bias(x, Wq, Wk):
    """Estimate the global score scale on a row sample; returns exp bias C."""
    x = np.asarray(x, np.float32)
    xs = x[:, ::16, :].reshape(-1, E)           # 256 sampled rows
    qs = xs @ np.asarray(Wq, np.float32).T
    ks = xs @ np.asarray(Wk, np.float32).T
    m = 0.0
    for b in range(B):
        qb = qs[b * 128 : (b + 1) * 128].reshape(128, H, D)
        kb = ks[b * 128 : (b + 1) * 128].reshape(128, H, D)
        s = np.einsum("qhd,khd->hqk", qb, kb) / np.sqrt(np.float32(D))
        m = max(m, float(np.abs(s).max()))
    if m * 1.6 < 25.0:
        return 0.0
    return round(m * 1.3, 3)


def classify_mask(mask):
    mask = np.asarray(mask)
    m0 = mask[0, 0]
    ar = np.arange(S)
    tril = (ar[:, None] >= ar[None, :])
    if all((mask[b, 0] != 0).astype(bool).__eq__(tril).all() for b in range(B)):
        return "causal"
    if (mask != 0).all():
        return "full"
    return "other"


def _numpy_fallback(x, sin_emb, cos_emb, mask, Wq, Wk, Wv, Wo):
    x = np.asarray(x, np.float32)
    sin_emb = np.asarray(sin_emb, np.float32)
    cos_emb = np.asarray(cos_emb, np.float32)
    mask = np.asarray(mask)
    Wq, Wk, Wv, Wo = (np.asarray(w, np.float32) for w in (Wq, Wk, Wv, Wo))

    def rope(t):
        sin = sin_emb[None, :, None, :]
        cos = cos_emb[None, :, None, :]
        x1 = t[..., 0::2]
        x2 = t[..., 1::2]
        r0 = x1 * cos - x2 * sin
        r1 = x1 * sin + x2 * cos
        return np.stack((r0, r1), axis=-1).reshape(t.shape)

    q = rope((x @ Wq.T).reshape(B, S, H, D))
    k = rope((x @ Wk.T).reshape(B, S, H, D))
    v = (x @ Wv.T).reshape(B, S, H, D)
    scores = np.einsum("bqhd,bkhd->bhqk", q, k) / np.sqrt(np.float32(D))
    scores = np.where(mask == 0, -np.inf, scores)
    scores -= scores.max(axis=-1, keepdims=True)
    ex = np.exp(scores)
    attn = ex / ex.sum(axis=-1, keepdims=True)
    out = np.einsum("bhqk,bkhd->bqhd", attn, v).reshape(B, S, E)
    return (out @ Wo.T).astype(np.float32)


def kernel(x, sin_emb, cos_emb, mask, Wq, Wk, Wv, Wo):
    mode = classify_mask(mask)
    if mode == "other":
        return _numpy_fallback(x, sin_emb, cos_emb, mask, Wq, Wk, Wv, Wo)

    from concourse.bass_utils import run_bass_kernel_spmd

    c_bias = probe_bias(x, Wq, Wk)
    nc = _get_module(mode == "causal", c_bias)
    in_maps = host_prep(x, sin_emb, cos_emb, Wq, Wk, Wv, Wo)
    res = run_bass_kernel_spmd(nc, in_maps, core_ids=list(range(N_CORES)))
    out = np.zeros((B, S, E), np.float32)
    for c in range(N_CORES):
        b = c // HL
        out[b] += res.results[c]["OUT"]
    return out


# revision 14
# speedup vs baseline: 2.7754x; 1.1282x over previous
"""Sharded RoPE causal attention for 8 Trainium2 NeuronCores.

Problem: B=2, S=2048, E=1024, H=16 heads, D=64 head_dim.
Sharding: batch x head-group (2 batches x 4 groups of 4 heads = 8 cores).
Each core computes its batch's attention for its 4 heads and a partial
output projection (row-parallel Wo); the host sums the 4 partials per batch.

Per-core layout strategy (all matmul layouts prepped on host):
  - x is fed transposed (xT [E, S]) so QKV projections contract E on
    partitions directly.
  - q,k are computed transposed (qT [256, S]) and RoPE is applied in that
    layout. Wq/Wk rows are pre-permuted per head into [even dims | odd dims]
    32-blocks so the RoPE pair partner is a fixed 32-partition block swap,
    done on the PE with a permutation matmul.
  - scores are computed transposed (sT[j,i] = k_j . q_i) per 128-row j-tile,
    causally skipping fully-masked column ranges; the diagonal 128x128 block
    gets -1e30 added below the diagonal before exp.
  - softmax denominators come for free from an appended ones-column on v
    (row 64 of the attn@v output is sum_j exp); normalization happens at
    eviction via gpsimd partition-broadcast + fast reciprocal + multiply.
  - exp is exp(s - C) with a global constant C picked on the host from a
    sampled probe of the score scale (C=0 for typical magnitudes); this is
    mathematically exact (cancels in softmax) and keeps fp32 exp in range.
"""

import sys

for _p in ("/opt/trn_rl_repo",):
    if _p not in sys.path:
        sys.path.insert(0, _p)

import numpy as np

B, S, E, H, D = 2, 2048, 1024, 16, 64
HL = 4          # heads per core
EL = HL * D     # 256: per-core slice of E
N_CORES = 8
NEG = -1e30

_module_cache = {}


def _patch_tile_drain():
    """This toolchain's walrus encodes at most 1 sem wait per instruction;
    Tile's closing drain carries one wait per used logical proc. Split the
    extra waits onto chained SP drains. (Compute-instruction waits are
    split by Bacc.generate_event_semaphores.)"""
    import concourse.tile as tile
    from concourse.vector_clock import ScopedClock

    if getattr(tile.TileContext, "_drain_split_patched", False):
        return

    def _drain_and_barrier(self, tick_clock, wait_clock):
        drain_inst = self.nc.sync.drain()
        wait_clock.add_sem_waits(
            drain_inst.ins, ScopedClock({None: tick_clock.global_clock})
        )
        si = drain_inst.ins.sync_info
        if si is not None and si.on_wait and len(si.on_wait) > 1:
            waits = list(si.on_wait)
            si.on_wait = waits[:1]
            for w in waits[1:]:
                extra = self.nc.sync.drain()
                xsi = extra.ins.sync_info
                if xsi is None:
                    import concourse.mybir as mybir

                    extra.ins.sync_info = mybir.SyncInfo(on_wait=[w], on_update=[])
                else:
                    xsi.on_wait = [w]
        self.nc.all_engine_barrier()
        assert self.sems is not None
        popped = self.nc._tile_sem_poison_stack.pop()
        assert popped is self._sem_poison
        self.nc.clear_and_free_semaphores(list(self.sems.allocated().values()))
        self.nc.all_engine_barrier()

    tile.TileContext._drain_and_barrier = _drain_and_barrier
    tile.TileContext._drain_split_patched = True


def build_module(causal: bool, c_bias: float, use_fp32r: bool = True):
    """Build the per-core Bass module (SPMD: same program on all 8 cores)."""
    _patch_tile_drain()
    from contextlib import ExitStack

    import concourse.tile as tile
    import concourse.mybir as mybir
    from concourse import bacc

    F32 = mybir.dt.float32
    AF = mybir.ActivationFunctionType
    OP = mybir.AluOpType

    nc = bacc.Bacc()

    # fp32 matmul is 4 cycles/row on TRN2; float32r streams at full rate.
    # The BIR verifier requires fp32r matmul operands to be *produced* as
    # float32r, so every matmul-operand tensor below is typed R32 (same bit
    # layout as fp32; numpy side stays float32).
    R32 = mybir.dt.float32r if use_fp32r else F32
    mm = nc.tensor.matmul

    NST = S // 128   # 16 s-tiles / j-tiles
    NEC = E // 128   # 8 e-chunks

    XT_d = nc.dram_tensor("XT", [E, S], R32, kind="ExternalInput")
    WQT_d = nc.dram_tensor("WQT", [E, EL], R32, kind="ExternalInput")
    WKT_d = nc.dram_tensor("WKT", [E, EL], R32, kind="ExternalInput")
    WVT_d = nc.dram_tensor("WVT", [E, EL], R32, kind="ExternalInput")
    WOT_d = nc.dram_tensor("WOT", [EL, E], R32, kind="ExternalInput")
    CC_d = nc.dram_tensor("CC", [128, S], F32, kind="ExternalInput")
    SS_d = nc.dram_tensor("SS", [128, S], F32, kind="ExternalInput")
    PM_d = nc.dram_tensor("PM", [128, 128], R32, kind="ExternalInput")
    TRI_d = nc.dram_tensor("TRI", [128, 128], R32, kind="ExternalInput")
    IDN_d = nc.dram_tensor("IDN", [128, 128], R32, kind="ExternalInput")
    VONES_d = nc.dram_tensor("VONES", [128, NST * HL], R32, kind="ExternalInput")
    OUT_d = nc.dram_tensor("OUT", [S, E], F32, kind="ExternalOutput")

    with tile.TileContext(nc) as tc, ExitStack() as ctx:
        consts = ctx.enter_context(tc.tile_pool(name="consts", bufs=1))
        CC = consts.tile([128, S], F32)
        SS = consts.tile([128, S], F32)
        PM = consts.tile([128, 128], R32)
        TRI = consts.tile([128, 128], R32)
        IDN = consts.tile([128, 128], R32)
        WOT = consts.tile([128, 2, E], R32)
        ebias = consts.tile([128, 1], F32)
        nc.vector.memset(ebias[:], -float(c_bias))

        qk = ctx.enter_context(tc.tile_pool(name="qk", bufs=1))
        QT = [qk.tile([128, S], R32, tag=f"qt{t}", name=f"qt{t}") for t in range(2)]
        KT = [qk.tile([128, S], R32, tag=f"kt{t}", name=f"kt{t}") for t in range(2)]
        VA = qk.tile([128, NST, HL, D + 1], R32, tag="vaug")  # col 0 = ones
        AN = [qk.tile([128, S], R32, tag=f"an{t}", name=f"an{t}") for t in range(2)]

        # ---------------- Phase 1: projections + RoPE ----------------
        with tc.tile_pool(name="xw", bufs=1) as xw, \
             tc.tile_pool(name="p1ps", bufs=4, space="PSUM") as p1ps, \
             tc.tile_pool(name="p1vps", bufs=2, space="PSUM") as p1vps, \
             tc.tile_pool(name="p1sw", bufs=2, space="PSUM") as p1sw, \
             tc.tile_pool(name="p1sb", bufs=2) as p1sb, \
             tc.tile_pool(name="p1t1", bufs=2) as p1t1:
            # DMA issue order = need order: q/k weights, then x chunks
            # (each e-chunk unblocks one accumulation step), cos/sin tables
            # for RoPE, remaining weights, then cold constants.
            WQTs = xw.tile([128, NEC, EL], R32, tag="wq")
            nc.sync.dma_start(out=WQTs[:], in_=WQT_d.rearrange("(c p) j -> p c j", p=128))
            WKTs = xw.tile([128, NEC, EL], R32, tag="wk")
            nc.sync.dma_start(out=WKTs[:], in_=WKT_d.rearrange("(c p) j -> p c j", p=128))
            # x arrives in (s-chunk, e-chunk) blocks so the first projection
            # accumulation group completes after ~2MB instead of all 8MB
            XTs = xw.tile([128, NEC, S], R32, tag="xt")
            for sc in range(4):
                for ec in range(NEC):
                    nc.sync.dma_start(
                        out=XTs[:, ec, sc * 512 : (sc + 1) * 512],
                        in_=XT_d[ec * 128 : (ec + 1) * 128, sc * 512 : (sc + 1) * 512],
                    )
            nc.sync.dma_start(out=CC[:], in_=CC_d[:])
            nc.sync.dma_start(out=SS[:], in_=SS_d[:])
            nc.sync.dma_start(out=PM[:], in_=PM_d[:])
            WVTs = xw.tile([128, NEC, EL], R32, tag="wv")
            nc.sync.dma_start(out=WVTs[:], in_=WVT_d.rearrange("(c p) j -> p c j", p=128))
            nc.sync.dma_start(out=TRI[:], in_=TRI_d[:])
            nc.sync.dma_start(out=IDN[:], in_=IDN_d[:])
            nc.sync.dma_start(
                out=VA[:, :, :, D : D + 1],
                in_=VONES_d.rearrange("p (st h) -> p st h", h=HL),
            )
            nc.sync.dma_start(out=WOT[:], in_=WOT_d.rearrange("(c p) e -> p c e", p=128))

            # q/k transposed projections + RoPE (chunked along s)
            # order: (q,k) tile 0 first so attention on heads 0/1 can begin,
            # then v, then (q,k) tile 1.
            for wten, dest, t in (
                (WQTs, QT, 0), (WKTs, KT, 0), (None, None, -1),
                (WQTs, QT, 1), (WKTs, KT, 1),
            ):
                if t < 0:
                    # v natural projection into VA
                    for st in range(NST):
                        pv = p1vps.tile([128, EL], F32, tag="pv")
                        for ec in range(NEC):
                            mm(
                                pv[:],
                                XTs[:, ec, st * 128 : (st + 1) * 128],
                                WVTs[:, ec, :],
                                start=(ec == 0),
                                stop=(ec == NEC - 1),
                            )
                        nc.scalar.copy(
                            VA[:, st, :, 0:D],
                            pv[:].rearrange("p (h d) -> p h d", h=HL),
                        )
                    continue
                if True:
                    for sc in range(4):
                        cs = slice(sc * 512, (sc + 1) * 512)
                        ps = p1ps.tile([128, 512], F32, tag="pqk")
                        for ec in range(NEC):
                            mm(
                                ps[:],
                                wten[:, ec, t * 128 : (t + 1) * 128],
                                XTs[:, ec, cs],
                                start=(ec == 0),
                                stop=(ec == NEC - 1),
                            )
                        raw = p1sb.tile([128, 512], R32, tag="raw", bufs=3)
                        nc.scalar.copy(raw[:], ps[:])
                        t1 = p1t1.tile([128, 512], F32, tag="t1")
                        nc.gpsimd.tensor_mul(t1[:], raw[:], CC[:, cs])
                        sw = p1sw.tile([128, 512], F32, tag="psw")
                        mm(sw[:], PM[:], raw[:], start=True, stop=True)
                        t2 = p1sb.tile([128, 512], F32, tag="t2")
                        nc.vector.tensor_mul(t2[:], sw[:], SS[:, cs])
                        nc.vector.tensor_add(dest[t][:, cs], t1[:], t2[:])


        # ---------------- Phase 2: attention ----------------
        with tc.tile_pool(name="sps", bufs=2, space="PSUM") as sps, \
             tc.tile_pool(name="aops", bufs=2, space="PSUM") as aops, \
             tc.tile_pool(name="ssb", bufs=3) as ssb, \
             tc.tile_pool(name="nrm", bufs=2) as nrm:
            pending = []
            for half in range(2):
                i0 = half * 1024
                jjs = list(range(8 * (half + 1))) if causal else list(range(NST))
                lastA = max(jj for jj in jjs
                            if (max(0, 128 * jj - i0) if causal else 0) < 512)
                lastB = jjs[-1]
                for t in range(2):
                    # interleave the pair of heads living in tile t so PE and
                    # ACT always have an independent stream to chew on
                    paos = [aops.tile([65, 1024], F32, tag="pao", name=f"pao{t}{hh}")
                            for hh in range(2)]
                    for jj in jjs:
                        if jj == 2 and pending:
                            for fn in pending:
                                fn()
                            pending.clear()
                        off = max(0, 128 * jj - i0) if causal else 0
                        pieces = [(off, 512), (512, 1024)] if off < 512 else [(off, 1024)]
                        for hh in range(2):
                            h = 2 * t + hh
                            r0 = 64 * hh
                            diag = causal and 8 * half <= jj < 8 * (half + 1)
                            sp = sps.tile([128, 1024], F32, tag="sp")
                            for (a, b) in pieces:
                                has_mask = diag and a == off
                                mm(
                                    sp[:, a:b],
                                    KT[t][r0 : r0 + 64, jj * 128 : (jj + 1) * 128],
                                    QT[t][r0 : r0 + 64, i0 + a : i0 + b],
                                    start=True,
                                    stop=not has_mask,
                                )
                                if has_mask:
                                    mm(
                                        sp[:, off : off + 128],
                                        IDN[:],
                                        TRI[:],
                                        start=False,
                                        stop=True,
                                    )
                            st_sb = ssb.tile([128, 1024], R32, tag="st")
                            nc.scalar.activation(
                                st_sb[:, off:1024], sp[:, off:1024], AF.Exp,
                                bias=ebias[:], scale=1.0,
                            )
                            for (a, b) in pieces:
                                mm(
                                    paos[hh][0:65, a:b],
                                    VA[:, jj, h, :],
                                    st_sb[:, a:b],
                                    start=(jj == 0),
                                    stop=(jj == (lastA if b == 512 else lastB)),
                                )
                    for bank in range(2):
                        cb = slice(bank * 512, (bank + 1) * 512)
                        for hh in range(2):
                            r0 = 64 * hh
                            u = nrm.tile([65, 512], F32, tag="u", bufs=6,
                                         name=f"u{t}{hh}{bank}")
                            nc.vector.tensor_copy(u[:], paos[hh][:, cb])
                            d0 = nrm.tile([1, 512], F32, tag="d0", bufs=6,
                                          name=f"d0{t}{hh}{bank}")
                            nc.vector.tensor_copy(d0[0:1, :], u[64:65, :])
                            bc = nrm.tile([64, 512], F32, tag="bc", bufs=6,
                                          name=f"bc{t}{hh}{bank}")
                            nc.gpsimd.partition_broadcast(bc[:], d0[0:1, :], channels=64)

                            def _finish(u=u, bc=bc, t=t, r0=r0, lo=i0 + bank * 512):
                                inv = nrm.tile([64, 512], F32, tag="inv", bufs=3,
                                               name="inv")
                                nc.vector.reciprocal_approx_fast(inv[:], bc[:])
                                nc.vector.tensor_mul(
                                    AN[t][r0 : r0 + 64, lo : lo + 512],
                                    u[0:64, :], inv[:],
                                )

                            pending.append(_finish)

            for fn in pending:
                fn()
            pending.clear()

        # ---------------- Phase 3: output projection ----------------
        with tc.tile_pool(name="ops", bufs=2, space="PSUM") as ops, \
             tc.tile_pool(name="osb", bufs=3) as osb:
            for st in range(NST):
                po = ops.tile([128, E], F32, tag="po")
                for eh in range(2):
                    for p in range(2):
                        mm(
                            po[:, eh * 512 : (eh + 1) * 512],
                            AN[p][:, st * 128 : (st + 1) * 128],
                            WOT[:, p, eh * 512 : (eh + 1) * 512],
                            start=(p == 0),
                            stop=(p == 1),
                        )
                ob = osb.tile([128, E], F32, tag="ob")
                nc.scalar.copy(ob[:], po[:])
                nc.sync.dma_start(out=OUT_d[st * 128 : (st + 1) * 128, :], in_=ob[:])

    nc.compile()
    return nc


def _get_module(causal: bool, c_bias: float):
    key = (causal, round(float(c_bias), 3))
    if key not in _module_cache:
        _module_cache[key] = build_module(causal, c_bias)
    return _module_cache[key]


_PERM64 = np.concatenate([np.arange(0, 64, 2), np.arange(1, 64, 2)])


def host_prep(x, sin_emb, cos_emb, Wq, Wk, Wv, Wo):
    """Build per-core input maps (host-side sharding + layout prep)."""
    x = np.asarray(x, np.float32)
    sin_emb = np.asarray(sin_emb, np.float32)
    cos_emb = np.asarray(cos_emb, np.float32)
    Wq = np.asarray(Wq, np.float32)
    Wk = np.asarray(Wk, np.float32)
    Wv = np.asarray(Wv, np.float32)
    Wo = np.asarray(Wo, np.float32)

    xT = [np.ascontiguousarray(x[b].T) for b in range(B)]

    cosT = np.ascontiguousarray(cos_emb.T)   # [32, S]
    sinT = np.ascontiguousarray(sin_emb.T)
    CC = np.ascontiguousarray(np.tile(cosT, (4, 1)))
    SSm = np.ascontiguousarray(
        np.concatenate([-sinT, sinT, -sinT, sinT], axis=0)
    )
    PM = np.zeros((128, 128), np.float32)
    for p in range(128):
        q = p + 32 if (p % 64) < 32 else p - 32
        PM[p, q] = 1.0
    TRIm = np.where(
        np.arange(128)[:, None] > np.arange(128)[None, :], np.float32(NEG), np.float32(0)
    ).astype(np.float32)

    scale = np.float32(1.0 / np.sqrt(D))
    in_maps = []
    for c in range(N_CORES):
        b, g = divmod(c, HL)
        rows = np.concatenate([(4 * g + lh) * 64 + _PERM64 for lh in range(HL)])
        in_maps.append({
            "XT": xT[b],
            "WQT": np.ascontiguousarray((Wq[rows, :] * scale).T),
            "WKT": np.ascontiguousarray(Wk[rows, :].T),
            "WVT": np.ascontiguousarray(Wv[256 * g : 256 * (g + 1), :].T),
            "WOT": np.ascontiguousarray(Wo[:, 256 * g : 256 * (g + 1)].T),
            "CC": CC,
            "VONES": np.ones((128, 64), np.float32),
            "SS": SSm,
            "PM": PM,
            "TRI": TRIm,
            "IDN": np.eye(128, dtype=np.float32),
        })
    return in_maps


def probe_bias(x, Wq, Wk):
    """Estimate the global score scale on a row sample; returns exp bias C."""
    x = np.asarray(x, np.float32)
    xs = x[:, ::16, :].reshape(-1, E)           # 256 sampled rows
    qs = xs @ np.asarray(Wq, np.float32).T
    ks = xs @ np.asarray(Wk, np.float32).T
    m = 0.0
    for b in range(B):
        qb = qs[b * 128 : (b + 1) * 128].reshape(128, H, D)
        kb = ks[b * 128 : (b + 1) * 128].reshape(128, H, D)
        s = np.einsum("qhd,khd->hqk", qb, kb) / np.sqrt(np.float32(D))
        m = max(m, float(np.abs(s).max()))
    if m * 1.6 < 25.0:
        return 0.0
    return round(m * 1.3, 3)


def classify_mask(mask):
    mask = np.asarray(mask)
    m0 = mask[0, 0]
    ar = np.arange(S)
    tril = (ar[:, None] >= ar[None, :])
    if all((mask[b, 0] != 0).astype(bool).__eq__(tril).all() for b in range(B)):
        return "causal"
    if (mask != 0).all():
        return "full"
    return "other"


def _numpy_fallback(x, sin_emb, cos_emb, mask, Wq, Wk, Wv, Wo):
    x = np.asarray(x, np.float32)
    sin_emb = np.asarray(sin_emb, np.float32)
    cos_emb = np.asarray(cos_emb, np.float32)
    mask = np.asarray(mask)
    Wq, Wk, Wv, Wo = (np.asarray(w, np.float32) for w in (Wq, Wk, Wv, Wo))

    def rope(t):
        sin = sin_emb[None, :, None, :]
        cos = cos_emb[None, :, None, :]
        x1 = t[..., 0::2]
        x2 = t[..., 1::2]
        r0 = x1 * cos - x2 * sin
        r1 = x1 * sin + x2 * cos
        return np.stack((r0, r1), axis=-1).reshape(t.shape)

    q = rope((x @ Wq.T).reshape(B, S, H, D))
    k = rope((x @ Wk.T).reshape(B, S, H, D))
    v = (x @ Wv.T).reshape(B, S, H, D)
    scores = np.einsum("bqhd,bkhd->bhqk", q, k) / np.sqrt(np.float32(D))
    scores = np.where(mask == 0, -np.inf, scores)
    scores -= scores.max(axis=-1, keepdims=True)
    ex = np.exp(scores)
    attn = ex / ex.sum(axis=-1, keepdims=True)
    out = np.einsum("bhqk,bkhd->bqhd", attn, v).reshape(B, S, E)
    return (out @ Wo.T).astype(np.float32)


def kernel(x, sin_emb, cos_emb, mask, Wq, Wk, Wv, Wo):
    mode = classify_mask(mask)
    if mode == "other":
        return _numpy_fallback(x, sin_emb, cos_emb, mask, Wq, Wk, Wv, Wo)

    from concourse.bass_utils import run_bass_kernel_spmd

    c_bias = probe_bias(x, Wq, Wk)
    nc = _get_module(mode == "causal", c_bias)
    in_maps = host_prep(x, sin_emb, cos_emb, Wq, Wk, Wv, Wo)
    res = run_bass_kernel_spmd(nc, in_maps, core_ids=list(range(N_CORES)))
    out = np.zeros((B, S, E), np.float32)
    for c in range(N_CORES):
        b = c // HL
        out[b] += res.results[c]["OUT"]
    return out
